# revision 25
# baseline (speedup 1.0000x reference)
"""Trainium2 Bass kernel for GQA sliding-window attention (8-core SPMD).

Problem: B=8, S=32, D=4096, H=32 Q-heads, KVH=8 KV-heads, HD=128,
sliding window 4096 with 4064 cached positions.

Sharding: tensor-parallel over heads. Core c owns Q heads 4c..4c+3 and KV
head c (one GQA group): Wq/Wk/Wv column-sharded, cache sharded by KV head,
x replicated. Attention runs in two head-pair passes; after each pass the
cores exchange that pass's (bf16) attention outputs peer-to-peer with
remote SBUF DMA broadcasts (all 8 cores share one TRN2 device), then each
core applies a column slice of Wo; the host concatenates column slices.

Layout/numerics notes:
  - All inputs are host-packed partition-major so every DMA lands with
    per-partition-contiguous multi-KB descriptor runs.
  - x is fed transposed (xT) so QKV projections produce Q^T/K^T directly
    in [head_dim, token] layout.
  - Wq/Wk columns (and cached K's hd axis) are permuted so RoPE's
    interleaved (even,odd) pairs become contiguous halves. The permutation
    cancels in q.k. SCALE is folded into Wq.
  - Peer exchange uses an XOR-permuted slot layout (core c stores peer
    p's data in slot c^p) so destination addresses are core-independent;
    the Wo row blocks are XOR-reordered per core on the host to match.
  - Softmax skips max-subtraction; normalization is deferred: row sums
    accumulate on the Vector engine (not the PE), and 1/sum is applied
    when copying attention outputs out of PSUM.
"""

import os
import sys
from contextlib import ExitStack

import numpy as np
import ml_dtypes

import concourse.bass as bass
import concourse.tile as tile
import concourse.mybir as mybir
from concourse import bacc
from concourse.bass_utils import run_bass_kernel_spmd
from concourse.masks import make_identity

BF16 = ml_dtypes.bfloat16

CORES = 8
B, S, D = 8, 32, 4096
H, KVH, HD = 32, 8, 128
SW = 4096
PREV = SW - S  # 4064
TOK = B * S  # 256
NH = H // KVH  # 4 Q heads per core
NHP = NH // 2  # head pairs per core
QCOLS = NH * HD  # 512 Q-projection columns per core
SCALE = float(HD) ** -0.5

# hd permutation: interleaved (r0,i0,r1,i1,...) -> (r..., i...)
_IDX = np.concatenate([np.arange(0, HD, 2), np.arange(1, HD, 2)])

# exec time of the last traced run (ns), set when KERNEL_TRACE=1
LAST_EXEC_NS = None

_BUILD_CACHE = {}


def _install_ntff_hook():
    """Register the axon NTFF profiling hook (the agent image's antenv stub
    lacks axon_hooks). Only needed when tracing."""
    import types

    if "antenv.axon_hooks" in sys.modules:
        return
    try:
        from trn_agent_boot.trn_boot import _ntff_profile_via_ctypes

        hook = _ntff_profile_via_ctypes("/opt/axon/libaxon_pjrt.so")
    except Exception:
        hook = None
    mod = types.ModuleType("antenv.axon_hooks")
    mod._hook = hook
    mod.get_axon_ntff_profile_hook = lambda: mod._hook
    mod.set_axon_ntff_profile_hook = lambda h: setattr(mod, "_hook", h)
    sys.modules["antenv.axon_hooks"] = mod
    import antenv

    antenv.axon_hooks = mod


def _bcast_rdests(j):
    """Slot list reaching relative peer j. Cross-die peers (bit 2 of the
    tpb delta) may only ride slots 4-7 (D2D-capable engines)."""
    if j & 4:
        return [None] * 4 + [(0, j)] * 4
    return [(0, j)] * 8


def _slot_of(j):
    """Measured on HW: cross-die deliveries land on peer (me^j^2), i.e. the
    D2D hop flips tpb bit 1. Store into the XOR-consistent slot so that slot
    s always holds data from core (me^s)."""
    return j ^ 2 if j & 4 else j


# remote_sem increments a receiver sees per pass: 16 per near peer
# (8 slots x 2), 8 per cross-die peer (4 slots x 2)
RSEM_PER_PASS = 4 * 16 + 4 * 8


def build(cores=CORES, gather=None, debug_taps=False):
    gather = gather or os.environ.get("KERNEL_GATHER", "cc")
    n_dc = D // 128  # 32 contraction chunks for QKV projections
    n_tc = (PREV + 127) // 128  # cache t-chunks (last short)
    tail = PREV - (n_tc - 1) * 128  # 96
    outc = D // cores  # Wo output columns per core
    n_xp = 4  # xt/wq DMA pieces
    xp = n_dc // n_xp

    dt = mybir.dt
    bf, f32 = dt.bfloat16, dt.float32
    EXP = mybir.ActivationFunctionType.Exp

    nc = bacc.Bacc("TRN2", target_bir_lowering=False, debug=False, num_devices=cores)

    xt_d = nc.dram_tensor("xt", [128, n_dc, TOK], bf, kind="ExternalInput")
    wq_d = nc.dram_tensor("wq", [128, n_dc, QCOLS], bf, kind="ExternalInput")
    wkv_d = nc.dram_tensor("wkv", [128, n_dc, 2 * HD], bf, kind="ExternalInput")
    kct_d = nc.dram_tensor("kct", [HD, PREV], bf, kind="ExternalInput")
    vc_d = nc.dram_tensor("vc", [128, n_tc, HD], bf, kind="ExternalInput")
    wo_d = nc.dram_tensor("wo", [128, H, outc], bf, kind="ExternalInput")
    cost_d = nc.dram_tensor("cost", [HD // 2, TOK], f32, kind="ExternalInput")
    sint_d = nc.dram_tensor("sint", [HD // 2, TOK], f32, kind="ExternalInput")
    maskt_d = nc.dram_tensor("maskt", [S, TOK], f32, kind="ExternalInput")
    out_d = nc.dram_tensor("out", [TOK, outc], f32, kind="ExternalOutput")
    if debug_taps:
        dbg = {
            "dbg_qt0": nc.dram_tensor("dbg_qt0", [128, 2, TOK], bf, kind="ExternalOutput"),
            "dbg_qt1": nc.dram_tensor("dbg_qt1", [128, 2, TOK], bf, kind="ExternalOutput"),
            "dbg_ktn": nc.dram_tensor("dbg_ktn", [128, TOK], bf, kind="ExternalOutput"),
            "dbg_vn": nc.dram_tensor("dbg_vn", [S, B, HD], bf, kind="ExternalOutput"),
            "dbg_ao0": nc.dram_tensor("dbg_ao0", [128, 2 * TOK], bf, kind="ExternalOutput"),
            "dbg_ao1": nc.dram_tensor("dbg_ao1", [128, 2 * TOK], bf, kind="ExternalOutput"),
            "dbg_allx0": nc.dram_tensor("dbg_allx0", [128, cores, 2, TOK], bf, kind="ExternalOutput"),
            "dbg_sum0": nc.dram_tensor("dbg_sum0", [128, 2 * TOK], bf, kind="ExternalOutput"),
        }

    with tile.TileContext(nc) as tc, ExitStack() as ctx:
        const = ctx.enter_context(tc.tile_pool(name="const", bufs=1))

        xt_sb = const.tile([128, n_dc, TOK], bf)
        wq_sb = const.tile([128, n_dc, QCOLS], bf)
        wkv_sb = const.tile([128, n_dc, 2 * HD], bf)
        kct_sb = const.tile([128, PREV], bf)
        vc_sb = const.tile([128, n_tc, HD], bf)
        wo_sb = const.tile([128, H, outc], bf)
        cost_sb = const.tile([HD // 2, TOK], f32)
        sint_sb = const.tile([HD // 2, TOK], f32)
        maskt_sb = const.tile([S, B, S], f32)
        ones_sb = const.tile([128, 1], bf)
        ident_sb = const.tile([128, 128], bf)
        qT_sb = [const.tile([128, 2, TOK], bf, tag=f"qT{p}", name=f"qT{p}") for p in range(NHP)]
        kTn_sb = const.tile([128, TOK], bf)
        vn_sb = const.tile([S, B, HD], bf)
        attn_new = [const.tile([S, 2, B, S], bf, tag=f"an{p}", name=f"an{p}") for p in range(NHP)]
        acc_sb = [const.tile([128, 2 * TOK], f32, tag=f"acc{p}", name=f"acc{p}") for p in range(NHP)]
        accb_sb = [const.tile([128, 2 * TOK], bf, tag=f"accb{p}", name=f"accb{p}") for p in range(NHP)]
        recip_sb = [const.tile([1, 2 * TOK], f32, tag=f"rc{p}", name=f"rc{p}") for p in range(NHP)]
        recip_bc = [const.tile([128, 2 * TOK], f32, tag=f"rb{p}", name=f"rb{p}") for p in range(NHP)]
        attnout = [const.tile([128, 2 * TOK], bf, tag=f"ao{p}", name=f"ao{p}") for p in range(NHP)]
        allx = [
            const.tile([128, cores, 2 * TOK], bf, tag=f"all{p}", name=f"all{p}")
            for p in range(NHP)
        ]
        out_sb = const.tile([128, 2, outc], f32, name="out_sb")
        warm_sb = const.tile([128, 512], bf, name="warm_sb")

        # ---- input DMAs: three HWDGE queues, first-needed first ----
        # sync queue: xt+wq pieces pace the projection; wkv right after.
        for i in range(n_xp):
            sl = slice(i * xp, (i + 1) * xp)
            nc.scalar.dma_start(out=xt_sb[:, sl, :], in_=xt_d.ap()[:, sl, :])
            nc.sync.dma_start(out=wq_sb[:, sl, :], in_=wq_d.ap()[:, sl, :])
        nc.sync.dma_start(out=wkv_sb[:], in_=wkv_d.ap())
        # scalar queue after xt: rope tables, KV cache, mask, then Wo (last)
        nc.scalar.dma_start(out=cost_sb[:], in_=cost_d.ap())
        nc.scalar.dma_start(out=sint_sb[:], in_=sint_d.ap())
        nc.scalar.dma_start(out=kct_sb[:], in_=kct_d.ap())
        nc.scalar.dma_start(out=vc_sb[:], in_=vc_d.ap())
        nc.scalar.dma_start(
            out=maskt_sb[:], in_=maskt_d.ap().rearrange("p (b s) -> p b s", b=B)
        )
        nc.scalar.dma_start(out=wo_sb[:], in_=wo_d.ap())

        # ---- on-device constants ----
        nc.gpsimd.memset(warm_sb[:], 0.0)
        nc.gpsimd.memset(ones_sb[:], 1.0)
        make_identity(nc, ident_sb[:])

        # ---- PE warmup: back-to-back matmuls push the HAM clock gate
        # toward full rate while input DMAs stream ----
        with tc.tile_pool(name="warm_ps", bufs=1, space="PSUM") as warm_pool:
            wps = warm_pool.tile([128, 512], f32, tag="wps", name="wps")
            for _ in range(14):
                nc.tensor.matmul(
                    wps[:], warm_sb[:, 0:128], warm_sb[:],
                    start=True, stop=True, skip_group_check=True,
                )

        # ---- peer-exchange setup ----
        # A tiny AllGather runs in both modes: with collectives configured,
        # the runtime launches all 8 cores in lockstep (without one, core
        # launches stagger by ~1.2ms and the first core's exec span eats
        # the skew waiting for peer data).
        dram = ctx.enter_context(tc.tile_pool(name="dram", bufs=1, space="DRAM"))
        agw_in = dram.tile([1, 64], bf, name="agw_in")
        agw_out = dram.tile([cores, 64], bf, name="agw_out", addr_space="Shared")
        # NOTE: the agw fill must NOT ride the gpsimd SWDGE queue — a
        # gpsimd.dma_start entry in the ring desyncs the prepare-only
        # bookkeeping that trigger_dma(count=None) fires for the remote
        # broadcasts, and the exchange then delivers garbage.
        nc.scalar.dma_start(out=agw_in[:], in_=warm_sb[0:1, 0:64])
        nc.gpsimd.collective_compute(
            "AllGather", mybir.AluOpType.bypass,
            replica_groups=[list(range(cores))],
            ins=[agw_in.opt()], outs=[agw_out.opt()],
        )
        if gather == "rdma":
            rsems = [nc.alloc_semaphore(f"rsem{p}") for p in range(NHP)]
            lsem = nc.alloc_semaphore("lsem")
        else:
            ag_in = [dram.tile([128, 2 * TOK], bf, tag=f"agi{p}", name=f"agi{p}") for p in range(NHP)]
            ag_out = [
                dram.tile([128 * cores, 2 * TOK], bf, tag=f"ago{p}", name=f"ago{p}",
                          addr_space="Shared")
                for p in range(NHP)
            ]

        rtmp = ctx.enter_context(tc.tile_pool(name="rope_tmp", bufs=4))

        def rope(src_ps, dst, tg):
            """src_ps/dst: [128, TOK]; partition halves are real/imag."""
            hh = HD // 2
            qr, qi = src_ps[0:hh, :], src_ps[hh:128, :]
            t1 = rtmp.tile([hh, TOK], f32, tag=f"{tg}1", name=f"{tg}1")
            t2 = rtmp.tile([hh, TOK], f32, tag=f"{tg}2", name=f"{tg}2")
            nc.vector.tensor_mul(t1[:], qr, cost_sb[:])
            nc.vector.tensor_mul(t2[:], qi, sint_sb[:])
            nc.vector.tensor_sub(dst[0:hh, :], t1[:], t2[:])
            t3 = rtmp.tile([hh, TOK], f32, tag=f"{tg}1", name=f"{tg}1")
            t4 = rtmp.tile([hh, TOK], f32, tag=f"{tg}2", name=f"{tg}2")
            nc.vector.tensor_mul(t3[:], qr, sint_sb[:])
            nc.vector.tensor_mul(t4[:], qi, cost_sb[:])
            nc.vector.tensor_add(dst[hh:128, :], t3[:], t4[:])

        # ---- phase 1: QKV projection, chunk-major ----
        # One PSUM bank per accumulator: the PE's start=True reset is
        # bank-wide, so co-locating two accumulation regions in one bank
        # wipes the partner's first chunk.
        with tc.tile_pool(name="proj_ps", bufs=1, space="PSUM") as proj_pool:
            q_ps = [proj_pool.tile([128, TOK], f32, tag=f"q{h}", name=f"q{h}")
                    for h in range(NH)]
            k_ps = proj_pool.tile([128, TOK], f32, tag="k", name="k")
            v_ps = proj_pool.tile([128, TOK], f32, tag="v", name="v")

            for c in range(n_dc):
                st, sp = c == 0, c == n_dc - 1
                x_c = xt_sb[:, c, :]
                nc.tensor.matmul(k_ps[:], wkv_sb[:, c, 0:HD], x_c,
                                 start=st, stop=sp, skip_group_check=True)
                nc.tensor.matmul(v_ps[:], wkv_sb[:, c, HD: 2 * HD], x_c,
                                 start=st, stop=sp, skip_group_check=True)
                for h in range(NH):
                    nc.tensor.matmul(q_ps[h][:], wq_sb[:, c, h * HD:(h + 1) * HD],
                                     x_c, start=st, stop=sp, skip_group_check=True)

            # V_new^T -> per-batch V_new [t=32, hd] via PE transpose
            vnT_sb = const.tile([128, TOK], bf, name="vnT")
            nc.scalar.copy(vnT_sb[:], v_ps[:])
            with tc.tile_pool(name="vt_ps", bufs=2, space="PSUM") as vt_pool:
                for b in range(B):
                    vt = vt_pool.tile([S, HD], bf, tag="vt", name="vt")
                    nc.tensor.transpose(vt[:], vnT_sb[:, b * S:(b + 1) * S], ident_sb[:])
                    nc.scalar.copy(vn_sb[:, b, :], vt[:])

            rope(q_ps[0][:], qT_sb[0][:, 0, :], "q")
            rope(q_ps[1][:], qT_sb[0][:, 1, :], "q")
            rope(k_ps[:], kTn_sb[:], "k")
            rope(q_ps[2][:], qT_sb[1][:, 0, :], "q")
            rope(q_ps[3][:], qT_sb[1][:, 1, :], "q")

        # ---- phase 2+3: attention in two head-pair passes ----
        s_pool = ctx.enter_context(tc.tile_pool(name="s_ps", bufs=3, space="PSUM"))
        acc_pool = ctx.enter_context(tc.tile_pool(name="acc_ps", bufs=1, space="PSUM"))
        attn_pool = ctx.enter_context(tc.tile_pool(name="attn", bufs=4))
        wo_pool = ctx.enter_context(tc.tile_pool(name="wo_ps", bufs=1, space="PSUM"))
        out_ps = [wo_pool.tile([128, outc], f32, tag=f"out{k}", name=f"out{k}") for k in range(2)]

        LOOK = 2
        for p in range(NHP):
            qpair = qT_sb[p][:, :, :]  # [128, 2, TOK]
            o_ps = acc_pool.tile([128, 2, TOK], f32, tag="o", name="o")
            sum_ps = acc_pool.tile([1, 2 * TOK], f32, tag="sum", name="sum")

            # cache chunk loop: PE does scores+AV; DVE accumulates row sums
            work = []
            def drain_one():
                pa, pn, pt = work.pop(0)
                nc.tensor.matmul(
                    sum_ps[0:1, :], ones_sb[0:pn, 0:1],
                    pa[0:pn].rearrange("p h t -> p (h t)"),
                    start=(pt == 0), stop=False, skip_group_check=True,
                )
                nc.tensor.matmul(
                    o_ps[:, :, :], vc_sb[0:pn, pt, :], pa[0:pn],
                    start=(pt == 0), stop=False, skip_group_check=True,
                )

            for t in range(n_tc):
                n = 128 if t < n_tc - 1 else tail
                s_ps = s_pool.tile([128, 2, TOK], f32, tag="s", name="s")
                nc.tensor.matmul(
                    s_ps[0:n, :, :], kct_sb[:, t * 128: t * 128 + n], qpair,
                    start=True, stop=True, skip_group_check=True,
                )
                a_sb = attn_pool.tile([128, 2, TOK], bf, tag="a", name="a")
                nc.scalar.activation(a_sb[0:n, :, :], s_ps[0:n, :, :], EXP)
                work.append((a_sb, n, t))
                if len(work) > LOOK:
                    drain_one()
            while work:
                drain_one()

            # new-token scores (t = prev..prev+S), all batches in one PSUM tile
            sn_ps = s_pool.tile([S, B, 2, S], f32, tag="s", name="sn")
            anp = attn_new[p][:, :, :, :]  # [S, 2, B, S]
            for b in range(B):
                nc.tensor.matmul(
                    sn_ps[0:S, b, :, :].rearrange("p h s -> p (h s)"),
                    kTn_sb[:, b * S:(b + 1) * S],
                    qpair[:, :, b * S:(b + 1) * S], start=True, stop=True,
                    skip_group_check=True,
                )
            nc.vector.tensor_tensor(
                out=sn_ps[:, :, :, :],
                in0=sn_ps[:, :, :, :],
                in1=maskt_sb[:].unsqueeze(2).broadcast_to((S, B, 2, S)),
                op=mybir.AluOpType.add,
            )
            nc.scalar.activation(
                anp.rearrange("p h b s -> p b h s"), sn_ps[:, :, :, :], EXP
            )
            # row sums: new-token part joins the cache accumulation
            nc.tensor.matmul(
                sum_ps[0:1, :], ones_sb[0:S, 0:1],
                anp.rearrange("p h b s -> p (h b s)"),
                start=False, stop=True, skip_group_check=True,
            )
            for b in range(B):
                for l in range(2):
                    nc.tensor.matmul(
                        o_ps[:, l, b * S:(b + 1) * S], vn_sb[:, b, :],
                        anp[:, l, b, :],
                        start=False, stop=(b == B - 1 and l == 1),
                        skip_group_check=True,
                    )

            # 1/rowsum -> broadcast -> normalize on PSUM->SBUF copy
            nc.vector.reciprocal_approx_fast(recip_sb[p][:], sum_ps[0:1, :])
            nc.gpsimd.partition_broadcast(recip_bc[p][:], recip_sb[p][:])
            norm = nc.vector.tensor_mul(
                attnout[p][:], o_ps[:, :, :].rearrange("p h t -> p (h t)"),
                recip_bc[p][:],
            )

            # peer exchange of this pass's attention outputs
            if gather == "rdma":
                from concourse.tile import add_dep_helper

                preps = []
                for j in range(cores):
                    preps.append(nc.gpsimd.remote_dma_broadcast(
                        out_ap=allx[p][:, _slot_of(j), :],
                        in_ap=attnout[p][:],
                        remote_sem=rsems[p], local_sem=lsem,
                        rdests=_bcast_rdests(j),
                    ))
                trig = nc.gpsimd.trigger_dma(count=None)
                # The trigger is a sequencer-level op: without explicit sync
                # edges it can fire before the Q7 engine has written all 8
                # preps' descriptors (observed on HW), and before the DVE
                # normalize has produced attnout. Wire both.
                add_dep_helper(trig.ins, norm.ins, sync=True,
                               reason="attnout ready before send")
                add_dep_helper(trig.ins, preps[-1].ins, sync=True,
                               reason="descriptors written before trigger")
            else:
                nc.scalar.dma_start(ag_in[p][:], attnout[p][:])
                nc.gpsimd.collective_compute(
                    "AllGather", mybir.AluOpType.bypass,
                    replica_groups=[list(range(cores))],
                    ins=[ag_in[p].opt()], outs=[ag_out[p].opt()],
                )
                eng = nc.sync if p == 0 else nc.scalar
                eng.dma_start(
                    allx[p][:],
                    ag_out[p].rearrange("(r p) n -> p r n", p=128),
                )

        # ---- phase 4: out = attnout_all @ Wo[:, slice], per pass ----
        # Wo rows are XOR-block-reordered on the host: block (j, p, l) holds
        # the head of core (mycore^j), pair p, member l. Pass-0's half runs
        # as soon as its exchange lands (while pass-1 attention streams);
        # pass-1's half waits only on its own exchange. The remote-arrival
        # waits (rsem) are invisible to the tile scheduler's single-core
        # sim — they are patched onto each PSUM bank chain's head matmul
        # after scheduling (see below).
        out_r = out_d.ap().rearrange("(k p) n -> p k n", p=128)
        wo_heads = []  # (instruction, rsem threshold)
        for p in range(NHP):
            for k in range(2):
                for j in range(cores):
                    for l in range(2):
                        g = j * NH + 2 * p + l
                        mm = nc.tensor.matmul(
                            out_ps[k][:],
                            allx[p][:, j, l * TOK + k * 128: l * TOK + k * 128 + 128],
                            wo_sb[:, g, :],
                            start=(p == 0 and j == 0 and l == 0),
                            stop=(p == NHP - 1 and j == cores - 1 and l == 1),
                            skip_group_check=True,
                        )
                        if j == 0 and l == 0 and gather == "rdma":
                            wo_heads.append((mm, rsems[p]))
                if p == NHP - 1:
                    nc.scalar.copy(out_sb[:, k, :], out_ps[k][:])
                    nc.sync.dma_start(out_r[:, k, :], out_sb[:, k, :])

        if debug_taps:
            nc.sync.dma_start(dbg["dbg_qt0"].ap(), qT_sb[0][:])
            nc.sync.dma_start(dbg["dbg_qt1"].ap(), qT_sb[1][:])
            nc.sync.dma_start(dbg["dbg_ktn"].ap(), kTn_sb[:])
            nc.sync.dma_start(dbg["dbg_vn"].ap(), vn_sb[:])
            nc.sync.dma_start(dbg["dbg_ao0"].ap(), attnout[0][:])
            nc.sync.dma_start(dbg["dbg_ao1"].ap(), attnout[1][:])
            nc.sync.dma_start(dbg["dbg_allx0"].ap().rearrange("p r (h t) -> p r h t", h=2), allx[0][:].rearrange("p r (h t) -> p r h t", h=2))
            nc.sync.dma_start(dbg["dbg_sum0"].ap(), accb_sb[0][:])

    if gather == "rdma":
        import bass_rust

        for mm, sem in wo_heads:
            ins = mm.ins
            si = ins.sync_info
            si.on_wait.append(
                bass_rust.SyncWait(
                    sync_type="semaphore", id=sem.num, ant_name=sem.name,
                    wait_mode="sem-ge-imm", wait_value=RSEM_PER_PASS, wait_reg=None,
                )
            )
            ins.sync_info = si

    nc.compile()
    return nc


def _pack(a):
    return np.ascontiguousarray(a)


def prep_in_maps(x, freqs_cos, freqs_sin, mask, cache_k, cache_v, Wq, Wk, Wv, Wo,
                 cores=CORES, gather="rdma"):
    """Host-side sharding + partition-major packing."""
    n_dc = D // 128
    n_tc = (PREV + 127) // 128
    outc = D // cores

    x = np.asarray(x, np.float32).reshape(TOK, D)
    xt = x.T.astype(BF16)  # [D, TOK]
    xt_p = _pack(xt.reshape(n_dc, 128, TOK).transpose(1, 0, 2))  # [128, n_dc, TOK]
    cost = _pack(np.tile(np.asarray(freqs_cos, np.float32)[0].T, (1, B)))
    sint = _pack(np.tile(np.asarray(freqs_sin, np.float32)[0].T, (1, B)))
    maskt = _pack(np.asarray(mask, np.float32).transpose(2, 0, 1).reshape(S, TOK))
    Wq = np.asarray(Wq, np.float32)
    Wk = np.asarray(Wk, np.float32)
    Wv = np.asarray(Wv, np.float32)
    Wo = np.asarray(Wo, np.float32)
    cache_k = np.asarray(cache_k, np.float32)
    cache_v = np.asarray(cache_v, np.float32)

    in_maps = []
    for c in range(cores):
        wq_c = (Wq[:, c * QCOLS:(c + 1) * QCOLS] * SCALE).reshape(D, NH, HD)[
            :, :, _IDX
        ].reshape(D, QCOLS).astype(BF16)
        wq_p = _pack(wq_c.reshape(n_dc, 128, QCOLS).transpose(1, 0, 2))
        wk_c = Wk[:, c * HD:(c + 1) * HD][:, _IDX]
        wv_c = Wv[:, c * HD:(c + 1) * HD]
        wkv_c = np.concatenate([wk_c, wv_c], axis=1).astype(BF16)
        wkv_p = _pack(wkv_c.reshape(n_dc, 128, 2 * HD).transpose(1, 0, 2))
        kct_c = _pack(cache_k[0, :PREV, c, :][:, _IDX].T.astype(BF16))  # [HD, PREV]
        vc_full = np.zeros((n_tc * 128, HD), np.float32)
        vc_full[:PREV] = cache_v[0, :PREV, c, :]
        vc_p = _pack(vc_full.astype(BF16).reshape(n_tc, 128, HD).transpose(1, 0, 2))
        # Wo rows XOR-reordered: block (j, p, l) = head 4*(c^j) + 2p + l
        wo_c = Wo[:, c * outc:(c + 1) * outc].astype(BF16)  # [H*HD, outc]
        wo_blocks = wo_c.reshape(H, HD, outc)
        src_of = (lambda j: c ^ j) if gather == "rdma" else (lambda j: j)
        order = [4 * src_of(j) + 2 * p + l
                 for j in range(cores) for p in range(NHP) for l in range(2)]
        wo_x = wo_blocks[order]  # [32, HD, outc]
        wo_p = _pack(wo_x.transpose(1, 0, 2))  # [128, H, outc]
        in_maps.append(
            {
                "xt": xt_p,
                "wq": wq_p,
                "wkv": wkv_p,
                "kct": kct_c,
                "vc": vc_p,
                "wo": wo_p,
                "cost": cost,
                "sint": sint,
                "maskt": maskt,
            }
        )
    return in_maps


def kernel(x, freqs_cos, freqs_sin, mask, cache_k, cache_v, Wq, Wk, Wv, Wo, positions):
    global LAST_EXEC_NS
    assert int(positions) == PREV, f"kernel compiled for positions={PREV}"

    key = ("v2", os.environ.get("KERNEL_GATHER", "cc"))
    if key not in _BUILD_CACHE:
        _BUILD_CACHE[key] = build(CORES, key[1])
    nc = _BUILD_CACHE[key]

    in_maps = prep_in_maps(
        x, freqs_cos, freqs_sin, mask, cache_k, cache_v, Wq, Wk, Wv, Wo,
        CORES, key[1]
    )

    trace = os.environ.get("KERNEL_TRACE", "0") == "1"
    if trace:
        _install_ntff_hook()
    res = run_bass_kernel_spmd(
        nc, in_maps, core_ids=list(range(CORES)), trace=trace
    )
    if trace:
        LAST_EXEC_NS = res.exec_time_ns

    outc = D // CORES
    out = np.empty((TOK, D), np.float32)
    for c in range(CORES):
        out[:, c * outc:(c + 1) * outc] = res.results[c]["out"]
    return out.reshape(B, S, D)


# revision 26
# speedup vs baseline: 1.0746x; 1.0746x over previous
"""Trainium2 Bass kernel for GQA sliding-window attention (8-core SPMD).

Problem: B=8, S=32, D=4096, H=32 Q-heads, KVH=8 KV-heads, HD=128,
sliding window 4096 with 4064 cached positions.

Sharding: tensor-parallel over heads. Core c owns Q heads 4c..4c+3 and KV
head c (one GQA group): Wq/Wk/Wv column-sharded, cache sharded by KV head,
x replicated. Attention runs in two head-pair passes; after each pass the
cores exchange that pass's (bf16) attention outputs peer-to-peer with
remote SBUF DMA broadcasts (all 8 cores share one TRN2 device), then each
core applies a column slice of Wo; the host concatenates column slices.

Layout/numerics notes:
  - All inputs are host-packed partition-major so every DMA lands with
    per-partition-contiguous multi-KB descriptor runs.
  - x is fed transposed (xT) so QKV projections produce Q^T/K^T directly
    in [head_dim, token] layout.
  - Wq/Wk columns (and cached K's hd axis) are permuted so RoPE's
    interleaved (even,odd) pairs become contiguous halves. The permutation
    cancels in q.k. SCALE is folded into Wq.
  - Peer exchange uses an XOR-permuted slot layout (core c stores peer
    p's data in slot c^p) so destination addresses are core-independent;
    the Wo row blocks are XOR-reordered per core on the host to match.
  - Softmax skips max-subtraction; normalization is deferred: row sums
    accumulate on the Vector engine (not the PE), and 1/sum is applied
    when copying attention outputs out of PSUM.
"""

import os
import sys
from contextlib import ExitStack

import numpy as np
import ml_dtypes

import concourse.bass as bass
import concourse.tile as tile
import concourse.mybir as mybir
from concourse import bacc
from concourse.bass_utils import run_bass_kernel_spmd
from concourse.masks import make_identity

BF16 = ml_dtypes.bfloat16

CORES = 8
B, S, D = 8, 32, 4096
H, KVH, HD = 32, 8, 128
SW = 4096
PREV = SW - S  # 4064
TOK = B * S  # 256
NH = H // KVH  # 4 Q heads per core
NHP = NH // 2  # head pairs per core
QCOLS = NH * HD  # 512 Q-projection columns per core
SCALE = float(HD) ** -0.5

# hd permutation: interleaved (r0,i0,r1,i1,...) -> (r..., i...)
_IDX = np.concatenate([np.arange(0, HD, 2), np.arange(1, HD, 2)])

# exec time of the last traced run (ns), set when KERNEL_TRACE=1
LAST_EXEC_NS = None

_BUILD_CACHE = {}


def _install_ntff_hook():
    """Register the axon NTFF profiling hook (the agent image's antenv stub
    lacks axon_hooks). Only needed when tracing."""
    import types

    if "antenv.axon_hooks" in sys.modules:
        return
    try:
        from trn_agent_boot.trn_boot import _ntff_profile_via_ctypes

        hook = _ntff_profile_via_ctypes("/opt/axon/libaxon_pjrt.so")
    except Exception:
        hook = None
    mod = types.ModuleType("antenv.axon_hooks")
    mod._hook = hook
    mod.get_axon_ntff_profile_hook = lambda: mod._hook
    mod.set_axon_ntff_profile_hook = lambda h: setattr(mod, "_hook", h)
    sys.modules["antenv.axon_hooks"] = mod
    import antenv

    antenv.axon_hooks = mod


def _bcast_rdests(j):
    """Slot list reaching relative peer j. Cross-die peers (bit 2 of the
    tpb delta) may only ride slots 4-7 (D2D-capable engines)."""
    if j & 4:
        return [None] * 4 + [(0, j)] * 4
    return [(0, j)] * 8


def _slot_of(j):
    """Measured on HW: cross-die deliveries land on peer (me^j^2), i.e. the
    D2D hop flips tpb bit 1. Store into the XOR-consistent slot so that slot
    s always holds data from core (me^s)."""
    return j ^ 2 if j & 4 else j


# remote_sem increments a receiver sees per pass: 16 per near peer
# (8 slots x 2), 8 per cross-die peer (4 slots x 2)
RSEM_PER_PASS = 4 * 16 + 4 * 8


def build(cores=CORES, gather=None, debug_taps=False):
    gather = gather or os.environ.get("KERNEL_GATHER", "cc")
    n_dc = D // 128  # 32 contraction chunks for QKV projections
    n_tc = (PREV + 127) // 128  # cache t-chunks (last short)
    tail = PREV - (n_tc - 1) * 128  # 96
    outc = D // cores  # Wo output columns per core
    n_xp = 4  # xt/wq DMA pieces
    xp = n_dc // n_xp

    dt = mybir.dt
    bf, f32 = dt.bfloat16, dt.float32
    EXP = mybir.ActivationFunctionType.Exp

    nc = bacc.Bacc("TRN2", target_bir_lowering=False, debug=False, num_devices=cores)

    xt_d = nc.dram_tensor("xt", [128, n_dc, TOK], bf, kind="ExternalInput")
    wq_d = nc.dram_tensor("wq", [128, n_dc, QCOLS], bf, kind="ExternalInput")
    wkv_d = nc.dram_tensor("wkv", [128, n_dc, 2 * HD], bf, kind="ExternalInput")
    kct_d = nc.dram_tensor("kct", [HD, PREV], bf, kind="ExternalInput")
    vc_d = nc.dram_tensor("vc", [128, n_tc, HD], bf, kind="ExternalInput")
    wo_d = nc.dram_tensor("wo", [128, H, outc], bf, kind="ExternalInput")
    cost_d = nc.dram_tensor("cost", [HD // 2, TOK], f32, kind="ExternalInput")
    sint_d = nc.dram_tensor("sint", [HD // 2, TOK], f32, kind="ExternalInput")
    maskt_d = nc.dram_tensor("maskt", [S, TOK], f32, kind="ExternalInput")
    out_d = nc.dram_tensor("out", [TOK, outc], f32, kind="ExternalOutput")
    if debug_taps:
        dbg = {
            "dbg_qt0": nc.dram_tensor("dbg_qt0", [128, 2, TOK], bf, kind="ExternalOutput"),
            "dbg_qt1": nc.dram_tensor("dbg_qt1", [128, 2, TOK], bf, kind="ExternalOutput"),
            "dbg_ktn": nc.dram_tensor("dbg_ktn", [128, TOK], bf, kind="ExternalOutput"),
            "dbg_vn": nc.dram_tensor("dbg_vn", [S, B, HD], bf, kind="ExternalOutput"),
            "dbg_ao0": nc.dram_tensor("dbg_ao0", [128, 2 * TOK], bf, kind="ExternalOutput"),
            "dbg_ao1": nc.dram_tensor("dbg_ao1", [128, 2 * TOK], bf, kind="ExternalOutput"),
            "dbg_allx0": nc.dram_tensor("dbg_allx0", [128, cores, 2, TOK], bf, kind="ExternalOutput"),
            "dbg_sum0": nc.dram_tensor("dbg_sum0", [128, 2 * TOK], bf, kind="ExternalOutput"),
        }

    with tile.TileContext(nc) as tc, ExitStack() as ctx:
        const = ctx.enter_context(tc.tile_pool(name="const", bufs=1))

        xt_sb = const.tile([128, n_dc, TOK], bf)
        wq_sb = const.tile([128, n_dc, QCOLS], bf)
        wkv_sb = const.tile([128, n_dc, 2 * HD], bf)
        kct_sb = const.tile([128, PREV], bf)
        vc_sb = const.tile([128, n_tc, HD], bf)
        wo_sb = const.tile([128, H, outc], bf)
        cost_sb = const.tile([HD // 2, TOK], f32)
        sint_sb = const.tile([HD // 2, TOK], f32)
        maskt_sb = const.tile([S, B, S], f32)
        ones_sb = const.tile([128, 1], bf)
        ident_sb = const.tile([128, 128], bf)
        qT_sb = [const.tile([128, 2, TOK], bf, tag=f"qT{p}", name=f"qT{p}") for p in range(NHP)]
        kTn_sb = const.tile([128, TOK], bf)
        vn_sb = const.tile([S, B, HD], bf)
        attn_new = [const.tile([S, 2, B, S], bf, tag=f"an{p}", name=f"an{p}") for p in range(NHP)]
        acc_sb = [const.tile([128, 2 * TOK], f32, tag=f"acc{p}", name=f"acc{p}") for p in range(NHP)]
        accb_sb = [const.tile([128, 2 * TOK], bf, tag=f"accb{p}", name=f"accb{p}") for p in range(NHP)]
        recip_sb = [const.tile([1, 2 * TOK], f32, tag=f"rc{p}", name=f"rc{p}") for p in range(NHP)]
        recip_bc = [const.tile([128, 2 * TOK], f32, tag=f"rb{p}", name=f"rb{p}") for p in range(NHP)]
        attnout = [const.tile([128, 2 * TOK], bf, tag=f"ao{p}", name=f"ao{p}") for p in range(NHP)]
        allx = [
            const.tile([128, cores, 2 * TOK], bf, tag=f"all{p}", name=f"all{p}")
            for p in range(NHP)
        ]
        out_sb = const.tile([128, 2, outc], f32, name="out_sb")
        warm_sb = const.tile([128, 512], bf, name="warm_sb")

        # ---- input DMAs: three HWDGE queues, first-needed first ----
        # sync queue: xt+wq pieces pace the projection; wkv right after.
        for i in range(n_xp):
            sl = slice(i * xp, (i + 1) * xp)
            nc.scalar.dma_start(out=xt_sb[:, sl, :], in_=xt_d.ap()[:, sl, :])
            nc.sync.dma_start(out=wq_sb[:, sl, :], in_=wq_d.ap()[:, sl, :])
        nc.sync.dma_start(out=wkv_sb[:], in_=wkv_d.ap())
        # scalar queue after xt: rope tables, KV cache, mask, then Wo (last)
        nc.scalar.dma_start(out=cost_sb[:], in_=cost_d.ap())
        nc.scalar.dma_start(out=sint_sb[:], in_=sint_d.ap())
        nc.scalar.dma_start(out=kct_sb[:], in_=kct_d.ap())
        nc.scalar.dma_start(out=vc_sb[:], in_=vc_d.ap())
        nc.scalar.dma_start(
            out=maskt_sb[:], in_=maskt_d.ap().rearrange("p (b s) -> p b s", b=B)
        )
        nc.scalar.dma_start(out=wo_sb[:], in_=wo_d.ap())

        # ---- on-device constants ----
        nc.gpsimd.memset(warm_sb[:], 0.0)
        nc.gpsimd.memset(ones_sb[:], 1.0)
        make_identity(nc, ident_sb[:])

        # ---- PE warmup: back-to-back matmuls push the HAM clock gate
        # toward full rate while input DMAs stream ----
        with tc.tile_pool(name="warm_ps", bufs=1, space="PSUM") as warm_pool:
            wps = warm_pool.tile([128, 512], f32, tag="wps", name="wps")
            for _ in range(14):
                nc.tensor.matmul(
                    wps[:], warm_sb[:, 0:128], warm_sb[:],
                    start=True, stop=True, skip_group_check=True,
                )

        # ---- peer-exchange setup ----
        # A tiny AllGather runs in both modes: with collectives configured,
        # the runtime launches all 8 cores in lockstep (without one, core
        # launches stagger by ~1.2ms and the first core's exec span eats
        # the skew waiting for peer data).
        dram = ctx.enter_context(tc.tile_pool(name="dram", bufs=1, space="DRAM"))
        agw_in = dram.tile([1, 64], bf, name="agw_in")
        agw_out = dram.tile([cores, 64], bf, name="agw_out", addr_space="Shared")
        # NOTE: the agw fill must NOT ride the gpsimd SWDGE queue — a
        # gpsimd.dma_start entry in the ring desyncs the prepare-only
        # bookkeeping that trigger_dma(count=None) fires for the remote
        # broadcasts, and the exchange then delivers garbage.
        nc.scalar.dma_start(out=agw_in[:], in_=warm_sb[0:1, 0:64])
        nc.gpsimd.collective_compute(
            "AllGather", mybir.AluOpType.bypass,
            replica_groups=[list(range(cores))],
            ins=[agw_in.opt()], outs=[agw_out.opt()],
        )
        if gather == "rdma":
            rsems = [nc.alloc_semaphore(f"rsem{p}") for p in range(NHP)]
            lsem = nc.alloc_semaphore("lsem")
        else:
            ag_in = [dram.tile([128, 2 * TOK], bf, tag=f"agi{p}", name=f"agi{p}") for p in range(NHP)]
            ag_out = [
                dram.tile([128 * cores, 2 * TOK], bf, tag=f"ago{p}", name=f"ago{p}",
                          addr_space="Shared")
                for p in range(NHP)
            ]

        rtmp = ctx.enter_context(tc.tile_pool(name="rope_tmp", bufs=4))

        def rope(src_ps, dst, tg):
            """src_ps/dst: [128, TOK]; partition halves are real/imag."""
            hh = HD // 2
            qr, qi = src_ps[0:hh, :], src_ps[hh:128, :]
            t1 = rtmp.tile([hh, TOK], f32, tag=f"{tg}1", name=f"{tg}1")
            t2 = rtmp.tile([hh, TOK], f32, tag=f"{tg}2", name=f"{tg}2")
            nc.vector.tensor_mul(t1[:], qr, cost_sb[:])
            nc.vector.tensor_mul(t2[:], qi, sint_sb[:])
            nc.vector.tensor_sub(dst[0:hh, :], t1[:], t2[:])
            t3 = rtmp.tile([hh, TOK], f32, tag=f"{tg}1", name=f"{tg}1")
            t4 = rtmp.tile([hh, TOK], f32, tag=f"{tg}2", name=f"{tg}2")
            nc.vector.tensor_mul(t3[:], qr, sint_sb[:])
            nc.vector.tensor_mul(t4[:], qi, cost_sb[:])
            nc.vector.tensor_add(dst[hh:128, :], t3[:], t4[:])

        # ---- phase 1: QKV projection, chunk-major ----
        # One PSUM bank per accumulator: the PE's start=True reset is
        # bank-wide, so co-locating two accumulation regions in one bank
        # wipes the partner's first chunk.
        with tc.tile_pool(name="proj_ps", bufs=1, space="PSUM") as proj_pool:
            q_ps = [proj_pool.tile([128, TOK], f32, tag=f"q{h}", name=f"q{h}")
                    for h in range(NH)]
            k_ps = proj_pool.tile([128, TOK], f32, tag="k", name="k")
            v_ps = proj_pool.tile([128, TOK], f32, tag="v", name="v")

            for c in range(n_dc):
                st, sp = c == 0, c == n_dc - 1
                x_c = xt_sb[:, c, :]
                nc.tensor.matmul(k_ps[:], wkv_sb[:, c, 0:HD], x_c,
                                 start=st, stop=sp, skip_group_check=True)
                nc.tensor.matmul(v_ps[:], wkv_sb[:, c, HD: 2 * HD], x_c,
                                 start=st, stop=sp, skip_group_check=True)
                for h in range(NH):
                    nc.tensor.matmul(q_ps[h][:], wq_sb[:, c, h * HD:(h + 1) * HD],
                                     x_c, start=st, stop=sp, skip_group_check=True)

            # V_new^T -> per-batch V_new [t=32, hd] via PE transpose
            vnT_sb = const.tile([128, TOK], bf, name="vnT")
            nc.scalar.copy(vnT_sb[:], v_ps[:])
            with tc.tile_pool(name="vt_ps", bufs=2, space="PSUM") as vt_pool:
                for b in range(B):
                    vt = vt_pool.tile([S, HD], bf, tag="vt", name="vt")
                    nc.tensor.transpose(vt[:], vnT_sb[:, b * S:(b + 1) * S], ident_sb[:])
                    nc.scalar.copy(vn_sb[:, b, :], vt[:])

            rope(q_ps[0][:], qT_sb[0][:, 0, :], "q")
            rope(q_ps[1][:], qT_sb[0][:, 1, :], "q")
            rope(k_ps[:], kTn_sb[:], "k")
            rope(q_ps[2][:], qT_sb[1][:, 0, :], "q")
            rope(q_ps[3][:], qT_sb[1][:, 1, :], "q")

        # ---- phase 2+3: attention in two head-pair passes ----
        s_pool = ctx.enter_context(tc.tile_pool(name="s_ps", bufs=3, space="PSUM"))
        acc_pool = ctx.enter_context(tc.tile_pool(name="acc_ps", bufs=1, space="PSUM"))
        attn_pool = ctx.enter_context(tc.tile_pool(name="attn", bufs=4))
        wo_pool = ctx.enter_context(tc.tile_pool(name="wo_ps", bufs=1, space="PSUM"))
        out_ps = [wo_pool.tile([128, outc], f32, tag=f"out{k}", name=f"out{k}") for k in range(2)]

        LOOK = 2
        for p in range(NHP):
            qpair = qT_sb[p][:, :, :]  # [128, 2, TOK]
            o_ps = acc_pool.tile([128, 2, TOK], f32, tag="o", name="o")
            sum_ps = acc_pool.tile([1, 2 * TOK], f32, tag="sum", name="sum")

            # cache chunk loop: PE does scores+AV; DVE accumulates row sums
            work = []
            def drain_one():
                pa, pn, pt = work.pop(0)
                nc.tensor.matmul(
                    sum_ps[0:1, :], ones_sb[0:pn, 0:1],
                    pa[0:pn].rearrange("p h t -> p (h t)"),
                    start=(pt == 0), stop=False, skip_group_check=True,
                )
                nc.tensor.matmul(
                    o_ps[:, :, :], vc_sb[0:pn, pt, :], pa[0:pn],
                    start=(pt == 0), stop=False, skip_group_check=True,
                )

            for t in range(n_tc):
                n = 128 if t < n_tc - 1 else tail
                s_ps = s_pool.tile([128, 2, TOK], f32, tag="s", name="s")
                nc.tensor.matmul(
                    s_ps[0:n, :, :], kct_sb[:, t * 128: t * 128 + n], qpair,
                    start=True, stop=True, skip_group_check=True,
                )
                a_sb = attn_pool.tile([128, 2, TOK], bf, tag="a", name="a")
                nc.scalar.activation(a_sb[0:n, :, :], s_ps[0:n, :, :], EXP)
                work.append((a_sb, n, t))
                if len(work) > LOOK:
                    drain_one()
            while work:
                drain_one()

            # new-token scores (t = prev..prev+S), all batches in one PSUM tile
            sn_ps = s_pool.tile([S, B, 2, S], f32, tag="s", name="sn")
            anp = attn_new[p][:, :, :, :]  # [S, 2, B, S]
            for b in range(B):
                nc.tensor.matmul(
                    sn_ps[0:S, b, :, :].rearrange("p h s -> p (h s)"),
                    kTn_sb[:, b * S:(b + 1) * S],
                    qpair[:, :, b * S:(b + 1) * S], start=True, stop=True,
                    skip_group_check=True,
                )
            nc.vector.tensor_tensor(
                out=sn_ps[:, :, :, :],
                in0=sn_ps[:, :, :, :],
                in1=maskt_sb[:].unsqueeze(2).broadcast_to((S, B, 2, S)),
                op=mybir.AluOpType.add,
            )
            nc.scalar.activation(
                anp.rearrange("p h b s -> p b h s"), sn_ps[:, :, :, :], EXP
            )
            # row sums: new-token part joins the cache accumulation
            nc.tensor.matmul(
                sum_ps[0:1, :], ones_sb[0:S, 0:1],
                anp.rearrange("p h b s -> p (h b s)"),
                start=False, stop=True, skip_group_check=True,
            )
            for b in range(B):
                for l in range(2):
                    nc.tensor.matmul(
                        o_ps[:, l, b * S:(b + 1) * S], vn_sb[:, b, :],
                        anp[:, l, b, :],
                        start=False, stop=(b == B - 1 and l == 1),
                        skip_group_check=True,
                    )

            # 1/rowsum -> broadcast -> normalize on PSUM->SBUF copy
            nc.vector.reciprocal_approx_fast(recip_sb[p][:], sum_ps[0:1, :])
            nc.gpsimd.partition_broadcast(recip_bc[p][:], recip_sb[p][:])
            norm = nc.vector.tensor_mul(
                attnout[p][:], o_ps[:, :, :].rearrange("p h t -> p (h t)"),
                recip_bc[p][:],
            )

            # peer exchange of this pass's attention outputs
            if gather == "rdma":
                from concourse.tile import add_dep_helper

                preps = []
                for j in range(cores):
                    preps.append(nc.gpsimd.remote_dma_broadcast(
                        out_ap=allx[p][:, _slot_of(j), :],
                        in_ap=attnout[p][:],
                        remote_sem=rsems[p], local_sem=lsem,
                        rdests=_bcast_rdests(j),
                    ))
                trig = nc.gpsimd.trigger_dma(count=None)
                # The trigger is a sequencer-level op: without explicit sync
                # edges it can fire before the Q7 engine has written all 8
                # preps' descriptors (observed on HW), and before the DVE
                # normalize has produced attnout. Wire both.
                add_dep_helper(trig.ins, norm.ins, sync=True,
                               reason="attnout ready before send")
                add_dep_helper(trig.ins, preps[-1].ins, sync=True,
                               reason="descriptors written before trigger")
            else:
                nc.scalar.dma_start(ag_in[p][:], attnout[p][:])
                nc.gpsimd.collective_compute(
                    "AllGather", mybir.AluOpType.bypass,
                    replica_groups=[list(range(cores))],
                    ins=[ag_in[p].opt()], outs=[ag_out[p].opt()],
                )
                # split the gathered readback across both HWDGE queues so
                # the Wo matmuls stop waiting on a single-queue 1MB drain
                ag_r = ag_out[p].rearrange("(r p) n -> p r n", p=128)
                nc.sync.dma_start(allx[p][:, 0:cores // 2, :], ag_r[:, 0:cores // 2, :])
                nc.scalar.dma_start(allx[p][:, cores // 2:, :], ag_r[:, cores // 2:, :])

        # ---- phase 4: out = attnout_all @ Wo[:, slice], per pass ----
        # Wo rows are XOR-block-reordered on the host: block (j, p, l) holds
        # the head of core (mycore^j), pair p, member l. Pass-0's half runs
        # as soon as its exchange lands (while pass-1 attention streams);
        # pass-1's half waits only on its own exchange. The remote-arrival
        # waits (rsem) are invisible to the tile scheduler's single-core
        # sim — they are patched onto each PSUM bank chain's head matmul
        # after scheduling (see below).
        out_r = out_d.ap().rearrange("(k p) n -> p k n", p=128)
        wo_heads = []  # (instruction, rsem threshold)
        for p in range(NHP):
            for k in range(2):
                for j in range(cores):
                    for l in range(2):
                        g = j * NH + 2 * p + l
                        mm = nc.tensor.matmul(
                            out_ps[k][:],
                            allx[p][:, j, l * TOK + k * 128: l * TOK + k * 128 + 128],
                            wo_sb[:, g, :],
                            start=(p == 0 and j == 0 and l == 0),
                            stop=(p == NHP - 1 and j == cores - 1 and l == 1),
                            skip_group_check=True,
                        )
                        if j == 0 and l == 0 and gather == "rdma":
                            wo_heads.append((mm, rsems[p]))
                if p == NHP - 1:
                    nc.scalar.copy(out_sb[:, k, :], out_ps[k][:])
                    nc.sync.dma_start(out_r[:, k, :], out_sb[:, k, :])

        if debug_taps:
            nc.sync.dma_start(dbg["dbg_qt0"].ap(), qT_sb[0][:])
            nc.sync.dma_start(dbg["dbg_qt1"].ap(), qT_sb[1][:])
            nc.sync.dma_start(dbg["dbg_ktn"].ap(), kTn_sb[:])
            nc.sync.dma_start(dbg["dbg_vn"].ap(), vn_sb[:])
            nc.sync.dma_start(dbg["dbg_ao0"].ap(), attnout[0][:])
            nc.sync.dma_start(dbg["dbg_ao1"].ap(), attnout[1][:])
            nc.sync.dma_start(dbg["dbg_allx0"].ap().rearrange("p r (h t) -> p r h t", h=2), allx[0][:].rearrange("p r (h t) -> p r h t", h=2))
            nc.sync.dma_start(dbg["dbg_sum0"].ap(), accb_sb[0][:])

    if gather == "rdma":
        import bass_rust

        for mm, sem in wo_heads:
            ins = mm.ins
            si = ins.sync_info
            si.on_wait.append(
                bass_rust.SyncWait(
                    sync_type="semaphore", id=sem.num, ant_name=sem.name,
                    wait_mode="sem-ge-imm", wait_value=RSEM_PER_PASS, wait_reg=None,
                )
            )
            ins.sync_info = si

    nc.compile()
    return nc


def _pack(a):
    return np.ascontiguousarray(a)


def prep_in_maps(x, freqs_cos, freqs_sin, mask, cache_k, cache_v, Wq, Wk, Wv, Wo,
                 cores=CORES, gather="rdma"):
    """Host-side sharding + partition-major packing."""
    n_dc = D // 128
    n_tc = (PREV + 127) // 128
    outc = D // cores

    x = np.asarray(x, np.float32).reshape(TOK, D)
    xt = x.T.astype(BF16)  # [D, TOK]
    xt_p = _pack(xt.reshape(n_dc, 128, TOK).transpose(1, 0, 2))  # [128, n_dc, TOK]
    cost = _pack(np.tile(np.asarray(freqs_cos, np.float32)[0].T, (1, B)))
    sint = _pack(np.tile(np.asarray(freqs_sin, np.float32)[0].T, (1, B)))
    maskt = _pack(np.asarray(mask, np.float32).transpose(2, 0, 1).reshape(S, TOK))
    Wq = np.asarray(Wq, np.float32)
    Wk = np.asarray(Wk, np.float32)
    Wv = np.asarray(Wv, np.float32)
    Wo = np.asarray(Wo, np.float32)
    cache_k = np.asarray(cache_k, np.float32)
    cache_v = np.asarray(cache_v, np.float32)

    in_maps = []
    for c in range(cores):
        wq_c = (Wq[:, c * QCOLS:(c + 1) * QCOLS] * SCALE).reshape(D, NH, HD)[
            :, :, _IDX
        ].reshape(D, QCOLS).astype(BF16)
        wq_p = _pack(wq_c.reshape(n_dc, 128, QCOLS).transpose(1, 0, 2))
        wk_c = Wk[:, c * HD:(c + 1) * HD][:, _IDX]
        wv_c = Wv[:, c * HD:(c + 1) * HD]
        wkv_c = np.concatenate([wk_c, wv_c], axis=1).astype(BF16)
        wkv_p = _pack(wkv_c.reshape(n_dc, 128, 2 * HD).transpose(1, 0, 2))
        kct_c = _pack(cache_k[0, :PREV, c, :][:, _IDX].T.astype(BF16))  # [HD, PREV]
        vc_full = np.zeros((n_tc * 128, HD), np.float32)
        vc_full[:PREV] = cache_v[0, :PREV, c, :]
        vc_p = _pack(vc_full.astype(BF16).reshape(n_tc, 128, HD).transpose(1, 0, 2))
        # Wo rows XOR-reordered: block (j, p, l) = head 4*(c^j) + 2p + l
        wo_c = Wo[:, c * outc:(c + 1) * outc].astype(BF16)  # [H*HD, outc]
        wo_blocks = wo_c.reshape(H, HD, outc)
        src_of = (lambda j: c ^ j) if gather == "rdma" else (lambda j: j)
        order = [4 * src_of(j) + 2 * p + l
                 for j in range(cores) for p in range(NHP) for l in range(2)]
        wo_x = wo_blocks[order]  # [32, HD, outc]
        wo_p = _pack(wo_x.transpose(1, 0, 2))  # [128, H, outc]
        in_maps.append(
            {
                "xt": xt_p,
                "wq": wq_p,
                "wkv": wkv_p,
                "kct": kct_c,
                "vc": vc_p,
                "wo": wo_p,
                "cost": cost,
                "sint": sint,
                "maskt": maskt,
            }
        )
    return in_maps


def kernel(x, freqs_cos, freqs_sin, mask, cache_k, cache_v, Wq, Wk, Wv, Wo, positions):
    global LAST_EXEC_NS
    assert int(positions) == PREV, f"kernel compiled for positions={PREV}"

    key = ("v2", os.environ.get("KERNEL_GATHER", "cc"))
    if key not in _BUILD_CACHE:
        _BUILD_CACHE[key] = build(CORES, key[1])
    nc = _BUILD_CACHE[key]

    in_maps = prep_in_maps(
        x, freqs_cos, freqs_sin, mask, cache_k, cache_v, Wq, Wk, Wv, Wo,
        CORES, key[1]
    )

    trace = os.environ.get("KERNEL_TRACE", "0") == "1"
    if trace:
        _install_ntff_hook()
    res = run_bass_kernel_spmd(
        nc, in_maps, core_ids=list(range(CORES)), trace=trace
    )
    if trace:
        LAST_EXEC_NS = res.exec_time_ns

    outc = D // CORES
    out = np.empty((TOK, D), np.float32)
    for c in range(CORES):
        out[:, c * outc:(c + 1) * outc] = res.results[c]["out"]
    return out.reshape(B, S, D)
